# revision 37
# baseline (speedup 1.0000x reference)
"""Trainium2 Bass kernel for ConvTemporalGraphical-style gated graph conv.

Computation (see reference):
    g   = x.reshape(N, F)                       # F = C*T*V = 204800
    h0  = elu(g @ W0 + b0)                      # [N, 256]   <-- dominant cost
    h1  = elu(h0 @ W1 + b1)                     # [N, 256]
    w   = softmax(h1 @ W2 + b2)                 # [N, 4]
    AS  = einsum('ne,etvw->ntvw', w, A)         # [N, T, V, V]
    out = einsum('nctv,ntvw->nctw', x, AS)

Sharding across 8 NeuronCores (one chip):
  * The F (contraction) dim of the big gating matmul is split 8 ways: core c
    holds W0 rows [c*25600, (c+1)*25600) (26 MB instead of 210 MB) and the
    matching slice of x, producing a partial h0 [32, 256].
  * A tiny AllReduce (32 KB) combines the partials; every core then runs the
    small MLP + softmax redundantly for all 32 samples.
  * The mixture + graph conv is data-parallel: core c owns samples
    [4c, 4c+4), selected on-device via a per-core one-hot matrix so all
    cores run the same graph (SPMD).

Device-friendly input layouts are produced on the host while sharding:
  * xgT: the gating x slice pre-transposed to [128, 200, 32] bf16 k-chunks
    (contraction dim on partitions), so no on-device transposes are needed.
  * W0s: bf16 [25600, 256]; the 128-row k-chunks are DMA'd directly as
    matmul moving operands.  bf16 halves HBM traffic for the dominant
    tensor; fp32 PSUM accumulation keeps the end-to-end error ~4e-4.
  * xcT / A4p: conv-side tensors pre-arranged into a v-padded layout
    (partition = 32*b + v with t = 32*b + g) so the 25x25(x64) graph-conv
    matmuls can be packed 4-at-a-time into the PE array via `tile_position`
    row groups, with samples paired on PSUM partition halves (col groups)
    so output DMAs use all 128 partitions.
"""

import sys

if "/opt/trn_rl_repo" not in sys.path:
    sys.path.insert(0, "/opt/trn_rl_repo")

import numpy as np

import concourse.bass as bass
import concourse.mybir as mybir
import concourse.tile as tile
from concourse import bacc
from concourse import bass_utils
from concourse.masks import make_identity

# Problem dims (hardcoded per contract).
N, C, T, V = 32, 64, 128, 25
F = C * T * V            # 204800
H = 256
E = 4
NCORES = 8
KS = F // NCORES         # 25600 rows of W0 per core
NLOC = N // NCORES       # 4 samples per core (conv slice)
KCH = KS // 128          # 200 k-chunks of 128 per core
TG = T // 4              # 32 t-groups; t = 32*b + g (b = row block, g = group)
W0GRP = 20               # k-chunks per W0 load

FP32 = mybir.dt.float32
BF16 = mybir.dt.bfloat16
AX = mybir.AxisListType
ALU = mybir.AluOpType
ACTF = mybir.ActivationFunctionType

CFG = {
    "gating_dtype": "bf16",   # "bf16" | "f32"
    "conv_dtype": "f32",      # "bf16" | "f32"  (conv feeds the output directly)
    "conv_pair_cols": True,   # odd samples on PSUM col group 64
    "phase": 5,
}


def _gdt():
    return BF16 if CFG["gating_dtype"] == "bf16" else FP32


def _cdt():
    return BF16 if CFG["conv_dtype"] == "bf16" else FP32


def build():
    nc = bacc.Bacc("TRN2", target_bir_lowering=False, debug=False, num_devices=NCORES)

    gdt = _gdt()
    xgT = nc.dram_tensor("xgT", [128, KCH, N], gdt, kind="ExternalInput")
    xcT = nc.dram_tensor("xcT", [128, NLOC // 2, TG, 2 * C], _cdt(),
                         kind="ExternalInput")
    W0s = nc.dram_tensor("W0s", [KS, H], gdt, kind="ExternalInput")
    b0 = nc.dram_tensor("b0", [H], FP32, kind="ExternalInput")
    W1 = nc.dram_tensor("W1", [H, H], FP32, kind="ExternalInput")
    b1 = nc.dram_tensor("b1", [H], FP32, kind="ExternalInput")
    W2 = nc.dram_tensor("W2", [H, E], FP32, kind="ExternalInput")
    b2 = nc.dram_tensor("b2", [E], FP32, kind="ExternalInput")
    A4p = nc.dram_tensor("A4p", [128, E, TG * V], _cdt(), kind="ExternalInput")
    selT = nc.dram_tensor("selT", [N, NLOC], FP32, kind="ExternalInput")
    out = nc.dram_tensor("out", [NLOC, C, T * V], FP32, kind="ExternalOutput")

    with tile.TileContext(nc) as tc:
        _build_body(nc, tc, xgT, xcT, W0s, b0, W1, b1, W2, b2, A4p, selT, out)
    nc.compile()
    return nc


def _build_body(nc, tc, xgT, xcT, W0s, b0, W1, b1, W2, b2, A4p, selT, out):
    from contextlib import ExitStack

    def _as_ap(t):
        return t if isinstance(t, bass.AP) else t.ap()

    xgT, xcT, W0s, b0, W1, b1, W2, b2, A4p, selT, out = map(
        _as_ap, (xgT, xcT, W0s, b0, W1, b1, W2, b2, A4p, selT, out)
    )
    gdt = _gdt()
    cdt = _cdt()

    ctx = ExitStack()
    with ctx:
        const = ctx.enter_context(tc.tile_pool(name="const", bufs=1))
        w0_pool = ctx.enter_context(tc.tile_pool(name="w0_pool", bufs=3))
        mix_pool = ctx.enter_context(tc.tile_pool(name="mix_pool", bufs=2))
        out_pool = ctx.enter_context(tc.tile_pool(name="out_pool", bufs=2))
        dram = ctx.enter_context(tc.tile_pool(name="dram", bufs=1, space="DRAM"))
        # PSUM bank budget (8): pg 1 + ph 2 + pc 4  (+1 spare)
        pg = ctx.enter_context(tc.tile_pool(name="pg", bufs=1, space="PSUM"))
        ph = ctx.enter_context(tc.tile_pool(name="ph", bufs=2, space="PSUM"))
        pc = ctx.enter_context(tc.tile_pool(name="pc", bufs=1, space="PSUM"))

        # ---- constants ----
        identity = const.tile([128, 128], FP32)
        make_identity(nc, identity)

        b0_row = const.tile([1, H], FP32)
        nc.sync.dma_start(b0_row[:], b0.rearrange("(o h) -> o h", o=1))
        b0b = const.tile([N, H], FP32)
        nc.gpsimd.partition_broadcast(b0b[:], b0_row[:])

        b1_row = const.tile([1, H], FP32)
        nc.scalar.dma_start(b1_row[:], b1.rearrange("(o h) -> o h", o=1))
        b1b = const.tile([N, H], FP32)
        nc.gpsimd.partition_broadcast(b1b[:], b1_row[:])

        b2_row = const.tile([1, E], FP32)
        nc.sync.dma_start(b2_row[:], b2.rearrange("(o h) -> o h", o=1))
        b2b = const.tile([N, E], FP32)
        nc.gpsimd.partition_broadcast(b2b[:], b2_row[:])

        W1_sb = const.tile([128, 2, H], FP32)
        nc.scalar.dma_start(W1_sb[:], W1.rearrange("(j p) h -> p j h", p=128))
        W2_sb = const.tile([128, 2, E], FP32)
        nc.sync.dma_start(W2_sb[:], W2.rearrange("(j p) h -> p j h", p=128))
        selT_sb = const.tile([N, NLOC], FP32)
        nc.scalar.dma_start(selT_sb[:], selT[:])

        # ---- persistent big SBUF tensors ----
        xT_all = const.tile([128, KCH, N], gdt)           # gating x^T chunks
        xcT_all = const.tile([128, NLOC // 2, TG, 2 * C], cdt)
        A_sb = const.tile([128, E, TG * V], cdt)          # padded A
        AS_sb = const.tile([128, NLOC, TG * V], cdt)      # mixture output

        # ---- bulk input loads (pre-transposed / pre-padded on host) ----
        nc.sync.dma_start(xT_all[:, :KCH // 2, :], xgT[:, :KCH // 2, :])
        nc.scalar.dma_start(xT_all[:, KCH // 2:, :], xgT[:, KCH // 2:, :])

        # =========================================================
        # Gating matmul: 200-chunk fp32-accumulated bf16 matmuls
        # =========================================================
        h0_ps = pg.tile([N, H], FP32)
        for g in range(KCH // W0GRP):
            w0_t = w0_pool.tile([128, W0GRP, H], gdt, tag="w0_t")
            w0_src = W0s.rearrange("(g j p) h -> g p j h", j=W0GRP, p=128)[g]
            dma_eng = nc.sync if g % 2 == 0 else nc.scalar
            dma_eng.dma_start(w0_t[:], w0_src)
            for j in range(W0GRP):
                k = g * W0GRP + j
                nc.tensor.matmul(
                    h0_ps[:],
                    xT_all[:, k, :],
                    w0_t[:, j, :],
                    start=(k == 0),
                    stop=(k == KCH - 1),
                )

        # conv-side loads, emitted late so they fill DMA gaps / the
        # collective wait rather than delaying the W0 stream.
        if CFG["phase"] >= 4:
            nc.sync.dma_start(xcT_all[:, 0], xcT[:, 0])
            nc.scalar.dma_start(xcT_all[:, 1], xcT[:, 1])
            nc.sync.dma_start(A_sb[:], A4p[:])

        if CFG["phase"] == 1:
            p1 = const.tile([N, H], FP32)
            nc.vector.tensor_copy(p1[:], h0_ps[:])
            nc.sync.dma_start(out[0][:N, :H], p1[:])
            return

        # =========================================================
        # Partial-h0 AllReduce (tiny; runs on TOPSP/SDMA silicon)
        # =========================================================
        h0p_sb = const.tile([N, H], FP32)
        nc.vector.tensor_copy(h0p_sb[:], h0_ps[:])
        cc_in = dram.tile([N, H], FP32)
        cc_out = dram.tile([N, H], FP32, addr_space="Shared")
        nc.gpsimd.dma_start(cc_in[:], h0p_sb[:])
        nc.gpsimd.collective_compute(
            "AllReduce",
            ALU.add,
            replica_groups=[list(range(NCORES))],
            ins=[cc_in.opt()],
            outs=[cc_out.opt()],
        )
        h0_sb = const.tile([N, H], FP32)
        nc.sync.dma_start(h0_sb[:], cc_out[:])
        if CFG["phase"] == 2:
            nc.sync.dma_start(out[0][:N, :H], h0_sb[:])
            return

        # =========================================================
        # Tiny MLP + softmax + local-w selection/broadcast
        # =========================================================
        def elu_inplace(t, width):
            tmp = const.tile([N, width], FP32, tag="elu_tmp", name="elu_tmp")
            nc.vector.tensor_scalar(tmp[:], t[:], 0.0, None, ALU.min)
            nc.scalar.activation(tmp[:], tmp[:], ACTF.Exp)
            nc.vector.tensor_scalar(t[:], t[:], 0.0, -1.0, ALU.max, ALU.add)
            nc.vector.tensor_tensor(t[:], t[:], tmp[:], ALU.add)

        nc.vector.tensor_tensor(h0_sb[:], h0_sb[:], b0b[:], ALU.add)
        elu_inplace(h0_sb, H)

        ps_h = ph.tile([128, 2 * N], FP32, tag="mlp_ps")
        for j in range(2):
            nc.tensor.transpose(
                ps_h[:, j * N:(j + 1) * N],
                h0_sb[:, j * 128:(j + 1) * 128],
                identity[:N, :N],
            )
        h0T = const.tile([128, 2, N], FP32)
        nc.vector.tensor_copy(h0T[:].rearrange("p j n -> p (j n)"), ps_h[:])

        h1_ps = ph.tile([N, H], FP32, tag="mlp_ps")
        for j in range(2):
            nc.tensor.matmul(
                h1_ps[:], h0T[:, j, :], W1_sb[:, j, :],
                start=(j == 0), stop=(j == 1),
            )
        h1_sb = const.tile([N, H], FP32)
        nc.vector.tensor_copy(h1_sb[:], h1_ps[:])
        nc.vector.tensor_tensor(h1_sb[:], h1_sb[:], b1b[:], ALU.add)
        elu_inplace(h1_sb, H)

        ps_h2 = ph.tile([128, 2 * N], FP32, tag="mlp_ps")
        for j in range(2):
            nc.tensor.transpose(
                ps_h2[:, j * N:(j + 1) * N],
                h1_sb[:, j * 128:(j + 1) * 128],
                identity[:N, :N],
            )
        h1T = const.tile([128, 2, N], FP32)
        nc.vector.tensor_copy(h1T[:].rearrange("p j n -> p (j n)"), ps_h2[:])

        lg_ps = ph.tile([N, E], FP32, tag="mlp_ps")
        for j in range(2):
            nc.tensor.matmul(
                lg_ps[:], h1T[:, j, :], W2_sb[:, j, :],
                start=(j == 0), stop=(j == 1),
            )
        lg_sb = const.tile([N, E], FP32)
        nc.vector.tensor_copy(lg_sb[:], lg_ps[:])
        nc.vector.tensor_tensor(lg_sb[:], lg_sb[:], b2b[:], ALU.add)

        # softmax over E (free dim)
        mx = const.tile([N, 1], FP32)
        nc.vector.reduce_max(mx[:], lg_sb[:], axis=AX.X)
        negmx = const.tile([N, 1], FP32)
        nc.vector.tensor_scalar_mul(negmx[:], mx[:], -1.0)
        ex = const.tile([N, E], FP32)
        sm = const.tile([N, 1], FP32)
        nc.scalar.activation(ex[:], lg_sb[:], ACTF.Exp, bias=negmx[:], accum_out=sm[:])
        rec = const.tile([N, 1], FP32)
        nc.vector.reciprocal(rec[:], sm[:])
        w_sb = const.tile([N, E], FP32)
        nc.vector.tensor_scalar(w_sb[:], ex[:], rec[:], None, ALU.mult)

        # local w: [4, 4] = selT^T @ w  (K = 32 on partitions)
        wl_ps = ph.tile([NLOC, E], FP32, tag="mlp_ps")
        nc.tensor.matmul(wl_ps[:], selT_sb[:], w_sb[:], start=True, stop=True)
        wloc = const.tile([NLOC, E], FP32)
        nc.vector.tensor_copy(wloc[:], wl_ps[:])

        # flatten [4, 4] -> [1, 16] (partition-crossing SBUF DMA), broadcast.
        w_row = const.tile([1, NLOC * E], FP32)
        nc.gpsimd.dma_start(
            w_row.rearrange("o (n e) -> o n e", n=NLOC), wloc[:]
        )
        w_bcast = const.tile([128, NLOC * E], FP32)
        nc.gpsimd.partition_broadcast(w_bcast[:], w_row[:])
        if CFG["phase"] == 3:
            nc.sync.dma_start(out[0][:, :NLOC * E], w_bcast[:C, :])
            return

        # =========================================================
        # Mixture AS[n] = sum_e w[n,e] * A[e]  (padded layout)
        # =========================================================
        for n in range(NLOC):
            acc = mix_pool.tile([128, TG * V], cdt, tag="mix_acc")
            tmp = mix_pool.tile([128, TG * V], cdt, tag="mix_tmp")
            nc.scalar.activation(
                acc[:], A_sb[:, 0, :], ACTF.Copy, scale=w_bcast[:, n * E:n * E + 1]
            )
            nc.vector.tensor_scalar(
                tmp[:], A_sb[:, 1, :], w_bcast[:, n * E + 1:n * E + 2], None, ALU.mult
            )
            nc.vector.tensor_tensor(acc[:], acc[:], tmp[:], ALU.add)
            nc.scalar.activation(
                tmp[:], A_sb[:, 2, :], ACTF.Copy, scale=w_bcast[:, n * E + 2:n * E + 3]
            )
            nc.vector.tensor_tensor(acc[:], acc[:], tmp[:], ALU.add)
            nc.vector.tensor_scalar(
                tmp[:], A_sb[:, 3, :], w_bcast[:, n * E + 3:n * E + 4], None, ALU.mult
            )
            nc.vector.tensor_tensor(AS_sb[:, n, :], acc[:], tmp[:], ALU.add)
        if CFG["phase"] == 4:
            nc.sync.dma_start(out[0][:, :TG * V], AS_sb[:C, 0, :])
            return

        # =========================================================
        # Graph conv: 25x25(x64) matmuls, 4-way row-packed, samples
        # paired on PSUM column groups.  t = 32*b + g.
        # =========================================================
        pair_cols = CFG["conv_pair_cols"]
        for pr in range(NLOC // 2):
            ot = out_pool.tile([128, T * V], FP32, tag="ot")
            for g0, glen in ((0, 20), (20, 12)):
                # width padded to 512 so the row stride is bank-aligned
                pob = [
                    pc.tile([128, 512], FP32, tag=f"po{b}", name=f"po{b}")
                    for b in range(4)
                ]
                for gi in range(glen):
                    g = g0 + gi
                    for b in range(4):
                        for j in range(2):
                            n = 2 * pr + j
                            nc.tensor.matmul(
                                pob[b][64 * j:64 * (j + 1),
                                       gi * V:(gi + 1) * V],
                                xcT_all[32 * b:32 * b + V, pr, g,
                                        64 * j:64 * (j + 1)],
                                AS_sb[32 * b:32 * b + V, n, g * V:(g + 1) * V],
                                start=True,
                                stop=True,
                                tile_position=(32 * b, 64 * j if pair_cols else 0),
                            )
                width = glen * V
                for b in range(4):
                    nc.vector.tensor_copy(
                        ot[:, (32 * b + g0) * V:(32 * b + g0) * V + width],
                        pob[b][:, :width],
                    )
                dma_eng = nc.sync if pr % 2 == 0 else nc.scalar
                od = out[2 * pr:2 * pr + 2].rearrange("n c f -> (n c) f")
                dma_eng.dma_start(
                    od.rearrange("r (b q) -> r b q", b=4)[:, :, g0 * V:g0 * V + width],
                    ot.rearrange("r (b q) -> r b q", b=4)[:, :, g0 * V:g0 * V + width],
                )


_NC_CACHE = {}


def _get_nc():
    key = (CFG["gating_dtype"], CFG["conv_dtype"], CFG["conv_pair_cols"], CFG["phase"])
    if key not in _NC_CACHE:
        _NC_CACHE[key] = build()
    return _NC_CACHE[key]


def _to_bf16(a):
    """Round-to-nearest-even fp32 -> bf16, vectorized."""
    import ml_dtypes

    u = np.ascontiguousarray(a, dtype=np.float32).view(np.uint32)
    r = ((u + 0x7FFF + ((u >> 16) & 1)) >> 16).astype(np.uint16)
    return r.view(ml_dtypes.bfloat16)


def _shard_inputs(x, W0, b0, W1, b1, W2, b2, A):
    x = np.ascontiguousarray(np.asarray(x, dtype=np.float32))
    W0 = np.ascontiguousarray(np.asarray(W0, dtype=np.float32))
    A = np.ascontiguousarray(np.asarray(A, dtype=np.float32)).reshape(E, T, V, V)
    xf = x.reshape(N, F)
    bf16 = CFG["gating_dtype"] == "bf16"
    cbf16 = CFG["conv_dtype"] == "bf16"

    # A in padded layout: A4p[32b+v, e, g*V+w] = A[e, 32b+g, v, w]
    A4p = np.zeros((128, E, TG * V), dtype=np.float32)
    At = A.reshape(E, 4, TG, V, V)            # e b g v w
    for b in range(4):
        A4p[32 * b:32 * b + V, :, :] = (
            At[:, b].transpose(2, 0, 1, 3).reshape(V, E, TG * V)
        )

    A4p_cast = _to_bf16(A4p) if cbf16 else A4p
    in_maps = []
    for c in range(NCORES):
        sel = np.zeros((N, NLOC), dtype=np.float32)
        for i in range(NLOC):
            sel[c * NLOC + i, i] = 1.0

        # gating slice, pre-transposed to [128, KCH, N]
        xg = xf[:, c * KS:(c + 1) * KS]                   # [N, KS]
        xgT = np.ascontiguousarray(
            xg.reshape(N, KCH, 128).transpose(2, 1, 0)    # [128, KCH, N]
        )
        if bf16:
            xgT = np.ascontiguousarray(_to_bf16(xgT))
            W0c = np.ascontiguousarray(_to_bf16(W0[c * KS:(c + 1) * KS]))
        else:
            W0c = np.ascontiguousarray(W0[c * KS:(c + 1) * KS])

        # conv slice, pre-transposed/padded:
        # xcT[32b+v, pr, g, 64j+cc] = x[4c + 2pr + j, cc, 32b+g, v]
        xl = x[c * NLOC:(c + 1) * NLOC]                   # [4, C, T, V]
        xcT = np.zeros((128, NLOC // 2, TG, 2 * C), dtype=np.float32)
        xr = xl.reshape(NLOC // 2, 2, C, 4, TG, V)        # pr j cc b g v
        for b in range(4):
            # [pr, j, cc, g, v] -> [v, pr, g, (j cc)]
            blk = xr[:, :, :, b]                          # pr j cc g v
            xcT[32 * b:32 * b + V] = (
                blk.transpose(4, 0, 3, 1, 2).reshape(V, NLOC // 2, TG, 2 * C)
            )

        in_maps.append({
            "xgT": xgT,
            "xcT": _to_bf16(xcT) if cbf16 else xcT,
            "W0s": W0c,
            "b0": np.asarray(b0, dtype=np.float32),
            "W1": np.asarray(W1, dtype=np.float32),
            "b1": np.asarray(b1, dtype=np.float32),
            "W2": np.asarray(W2, dtype=np.float32),
            "b2": np.asarray(b2, dtype=np.float32),
            "A4p": A4p_cast,
            "selT": sel,
        })
    return in_maps


def kernel(x, W0, b0, W1, b1, W2, b2, A):
    nc = _get_nc()
    in_maps = _shard_inputs(x, W0, b0, W1, b1, W2, b2, A)
    res = bass_utils.run_bass_kernel_spmd(nc, in_maps, core_ids=list(range(NCORES)))
    outs = [res.results[c]["out"].reshape(NLOC, C, T, V) for c in range(NCORES)]
    return np.concatenate(outs, axis=0)
